# revision 16
# baseline (speedup 1.0000x reference)
"""Trainium2 Bass kernel for DigitConvolutionalModel.

Model: x[B,784] -> reshape 28x28 -> 3x3 valid conv -> [B,676] -> FC(676,300)
       -> ReLU -> FC(300,10).

Strategy:
  * Fold the conv into FC1 on the host: feat @ w1 == x @ W1e where
    W1e[784,300] = C @ w1 (C = sparse conv scatter). Weight-only preprocessing.
  * Pure data parallel over 8 NeuronCores: batch shard of 8192 rows per core.
  * Per-core shard is passed pre-transposed (feature-major) so the contraction
    dim (784 = 7 chunks x 112) sits on SBUF partitions; the kernel computes
    transposed activations throughout (batch on the free axis):
        a1T[300,b] = relu(W1e.T @ xT + b1);  yT[10,b] = w2.T @ a1T
  * fp16 matmul operands (1 cyc/row at every PE p-state, half the HBM
    traffic of fp32) with fp32 PSUM accumulation; biases fp32.
  * SUBT=2 batch tiles are processed per weight-stationary step: each
    LDWEIGHTS serves SUBT matmuls (measured ~25 ns/MM less PE time than
    reloading weights per matmul).
  * Layer 2 is column-tiled: the three K=100 hidden chunks run as
    concurrent matmuls in PE column groups (0,32,64); the three partial
    yT outputs land on disjoint PSUM partitions (0-9, 32-41, 64-73), are
    copied to SBUF as one [74,BT] tile, stored with one DMA, and summed
    on the host during the gather step (b2 is added there too).
  * Layer 2 for a group is emitted between L1 j-chunks of the next group
    so the PE never waits on the ReLU (no PE gaps, stays at max p-state).
  * Output: ytp[74,8192] per core; host takes rows {0-9,32-41,64-73}.
"""

import os
import sys

sys.path.insert(0, "/opt/trn_rl_repo")

import numpy as np

import concourse.tile as tile
from concourse import bacc, mybir
from concourse.bass_utils import run_bass_kernel_spmd

# ---- problem constants (hardcoded per harness contract) ----
B = 65536
D = 784  # 28*28
H = 300
O = 10
IMG = 28
KH = KW = 3
OUT_HW = IMG - KH + 1  # 26

N_CORES = 8
BS = B // N_CORES  # 8192 rows per core

KCH = 7  # contraction chunks
KP = D // KCH  # 112 partitions per chunk
BT = int(os.environ.get("BT_SIZE", "512"))  # batch tile (512 = one PSUM bank)
NBT = BS // BT
MPAD = 128  # padded partition count for hidden-chunk tensors

# matmul operand dtype: f16 (default) | bf16 | f32r | f32
_MM_CHOICE = os.environ.get("BASS_MM_DT", "f16")
MM_DT = {
    "f32": mybir.dt.float32,
    "f32r": mybir.dt.float32r,
    "bf16": mybir.dt.bfloat16,
}.get(_MM_CHOICE, mybir.dt.float16)
if _MM_CHOICE in ("f32", "f32r"):
    MM_NP = np.float32
elif _MM_CHOICE == "bf16":
    import ml_dtypes

    MM_NP = ml_dtypes.bfloat16
else:
    MM_NP = np.float16

# hidden-dim chunking (sum must be H)
M_CHUNKS = {
    "even": [100, 100, 100],
    "fwl": [128, 128, 44],
}[os.environ.get("M_CHUNKS", "even")]
M_OFFS = [sum(M_CHUNKS[:i]) for i in range(len(M_CHUNKS))]
MCH = len(M_CHUNKS)

# tunables (env-overridable for experiments)
SUBT = int(os.environ.get("SUBT", "2"))  # batch tiles per weight-stationary step
XP_BUFS = int(os.environ.get("XP_BUFS", "5"))
AP_BUFS = int(os.environ.get("AP_BUFS", "4"))
PS1_BUFS = int(os.environ.get("PS1_BUFS", "3" if SUBT == 2 else "4"))
PS2_BUFS = int(os.environ.get("PS2_BUFS", "2"))
X_DMA_SPLIT = int(os.environ.get("X_DMA_SPLIT", "1"))  # k-chunk granularity of x loads
L2_COLTILE = os.environ.get("L2_COLTILE", "1") == "1"
L2_PIPELINE = os.environ.get("L2_PIPELINE", "1") == "1"
REPS = int(os.environ.get("KERNEL_REPS", "1"))  # timing only: repeat body in-module
SKIP_L2 = os.environ.get("SKIP_L2", "0") == "1"  # ablation: L2 on first group only
SKIP_X = os.environ.get("SKIP_X", "0") == "1"  # ablation: load x once, reuse
ACT_ENG = os.environ.get("ACT_ENG", "act")  # act | dve | pool | mix (relu engine)
YT_ENG = os.environ.get("YT_ENG", "dve")  # dve | pool | act (psum->sbuf copy)
X_DMA_ENG = os.environ.get("X_DMA_ENG", "sync")  # engine issuing x loads
YT_DMA_ENG = os.environ.get("YT_DMA_ENG", "sync")  # engine issuing yt stores
ACT_TINY = os.environ.get("ACT_TINY", "0") == "1"  # ablation: 1-partition act

# partial-output partition bases (column groups) when col-tiling
PB = [32 * j for j in range(MCH)] if L2_COLTILE else [0] * MCH
YP_P = PB[-1] + O if L2_COLTILE else O  # partitions used by ps2/yt tiles

_cache = {}


def _build_nc():
    f32 = mybir.dt.float32
    mdt = MM_DT

    nc = bacc.Bacc("TRN2", target_bir_lowering=False, debug=False, num_devices=N_CORES)
    xt_d = nc.declare_dram_parameter("xt", [KP, NBT, KCH, BT], mdt, isOutput=False)
    w1_d = nc.declare_dram_parameter("w1e", [KP, KCH * H], mdt, isOutput=False)
    b1_d = nc.declare_dram_parameter("b1r", [MPAD, MCH], f32, isOutput=False)
    w2_d = nc.declare_dram_parameter("w2r", [MPAD, MCH * O], mdt, isOutput=False)
    ytp_d = nc.declare_dram_parameter("ytp", [YP_P, BS], f32, isOutput=True)

    with tile.TileContext(nc) as tc:
        with (
            tc.tile_pool(name="singles", bufs=1) as singles,
            tc.tile_pool(name="xp", bufs=XP_BUFS) as xp,
            tc.tile_pool(name="ap", bufs=AP_BUFS) as ap,
            tc.tile_pool(name="yp", bufs=3) as yp,
            tc.tile_pool(name="ps1", bufs=PS1_BUFS, space="PSUM") as ps1p,
            tc.tile_pool(name="ps2", bufs=PS2_BUFS, space="PSUM") as ps2p,
        ):
            w1sb = singles.tile([KP, KCH * H], mdt)
            nc.sync.dma_start(w1sb[:], w1_d[:])
            b1sb = singles.tile([MPAD, MCH], f32)
            nc.sync.dma_start(b1sb[:], b1_d[:])
            w2sb = singles.tile([MPAD, MCH * O], mdt)
            nc.sync.dma_start(w2sb[:], w2_d[:])

            def _eng(name):
                return {
                    "sync": nc.sync,
                    "scalar": nc.scalar,
                    "vector": nc.vector,
                    "pool": nc.gpsimd,
                }[name]

            def load_x(bt, s, split):
                xt = xp.tile([KP, KCH, BT], mdt, name=f"xt{s}")
                step = (KCH + split - 1) // split if split > 1 else KCH
                for lo in range(0, KCH, step):
                    hi = min(lo + step, KCH)
                    _eng(X_DMA_ENG).dma_start(xt[:, lo:hi, :], xt_d[:, bt, lo:hi, :])
                return xt

            def layer2_store(a1, bt):
                ps2 = ps2p.tile([YP_P, BT], f32)
                for j in range(MCH):
                    mlen = M_CHUNKS[j]
                    if L2_COLTILE:
                        nc.tensor.matmul(
                            ps2[PB[j] : PB[j] + O, :],
                            w2sb[0:mlen, j * O : (j + 1) * O],
                            a1[0:mlen, j, :],
                            start=True,
                            stop=True,
                            tile_position=(0, PB[j]),
                        )
                    else:
                        nc.tensor.matmul(
                            ps2[:],
                            w2sb[0:mlen, j * O : (j + 1) * O],
                            a1[0:mlen, j, :],
                            start=(j == 0),
                            stop=(j == MCH - 1),
                        )
                yt = yp.tile([YP_P, BT], f32)
                yt_eng = {"dve": nc.vector, "pool": nc.gpsimd, "act": nc.scalar}[
                    YT_ENG
                ]
                if YT_ENG == "act":
                    yt_eng.copy(yt[:], ps2[:])
                else:
                    yt_eng.tensor_scalar_add(yt[:], ps2[:], 0.0)
                _eng(YT_DMA_ENG).dma_start(ytp_d[:, bt * BT : (bt + 1) * BT], yt[:])

            order = [i for _ in range(REPS) for i in range(NBT)]
            groups = [order[i : i + SUBT] for i in range(0, len(order), SUBT)]
            pending = []
            xts_cache = None
            for gi, bts in enumerate(groups):
                if SKIP_X and xts_cache is not None:
                    xts = xts_cache
                else:
                    xts = [
                        load_x(bt, s, split=(KCH if gi == 0 else X_DMA_SPLIT))
                        for s, bt in enumerate(bts)
                    ]
                    xts_cache = xts
                a1s = [
                    ap.tile([MPAD, MCH, BT], mdt, name=f"a1{s}")
                    for s in range(len(bts))
                ]
                for j in range(MCH):
                    mlen, moff = M_CHUNKS[j], M_OFFS[j]
                    pss = [
                        ps1p.tile([MPAD, BT], f32, name=f"ps{s}")
                        for s in range(len(bts))
                    ]
                    for k in range(KCH):
                        for s in range(len(bts)):
                            nc.tensor.matmul(
                                pss[s][0:mlen, :],
                                w1sb[:, k * H + moff : k * H + moff + mlen],
                                xts[s][:, k, :],
                                start=(k == 0),
                                stop=(k == KCH - 1),
                            )
                    for s in range(len(bts)):
                        if ACT_TINY:
                            nc.scalar.activation(
                                a1s[s][0:1, j, :],
                                pss[s][0:1, :],
                                mybir.ActivationFunctionType.Relu,
                            )
                            continue
                        eng = ACT_ENG
                        if eng == "mix":
                            eng = "act" if (j * SUBT + s) % 2 == 0 else "dve"
                        if eng == "act":
                            nc.scalar.activation(
                                a1s[s][0:mlen, j, :],
                                pss[s][0:mlen, :],
                                mybir.ActivationFunctionType.Relu,
                                bias=b1sb[0:mlen, j : j + 1],
                            )
                        else:
                            e = nc.vector if eng == "dve" else nc.gpsimd
                            e.tensor_scalar(
                                a1s[s][0:mlen, j, :],
                                pss[s][0:mlen, :],
                                b1sb[0:mlen, j : j + 1],
                                0.0,
                                mybir.AluOpType.add,
                                mybir.AluOpType.max,
                            )
                    if j == 0 and pending and L2_PIPELINE:
                        for p in pending:
                            layer2_store(*p)
                        pending = []
                if SKIP_L2 and gi > 0:
                    pass
                elif L2_PIPELINE:
                    pending = [(a1s[s], bts[s]) for s in range(len(bts))]
                else:
                    for s in range(len(bts)):
                        layer2_store(a1s[s], bts[s])
            for p in pending:
                layer2_store(*p)

    nc.compile()
    return nc


def _host_prep_weights(conv_w, w1, b1, w2):
    # Fold conv into FC1: W1e = C @ w1, computed in f64 then cast.
    w1g = w1.astype(np.float64).reshape(OUT_HW, OUT_HW, H)
    w1e = np.zeros((IMG, IMG, H), dtype=np.float64)
    cw = conv_w.astype(np.float64)
    for di in range(KH):
        for dj in range(KW):
            w1e[di : di + OUT_HW, dj : dj + OUT_HW, :] += cw[di, dj] * w1g
    w1e = w1e.reshape(D, H).astype(np.float32)

    w1e_r = np.ascontiguousarray(
        w1e.reshape(KCH, KP, H).transpose(1, 0, 2).reshape(KP, KCH * H)
    ).astype(MM_NP)
    b1f = b1.reshape(H)
    b1_r = np.zeros((MPAD, MCH), np.float32)
    w2_r = np.zeros((MPAD, MCH * O), MM_NP)
    for j in range(MCH):
        mlen, moff = M_CHUNKS[j], M_OFFS[j]
        b1_r[0:mlen, j] = b1f[moff : moff + mlen]
        w2_r[0:mlen, j * O : (j + 1) * O] = w2[moff : moff + mlen, :]
    return w1e_r, b1_r, w2_r


def _host_prep_x(xc):
    """Per-core shard [BS, 784] -> feature-major DRAM layout.

    xt[p, bt, k, b] = xc[bt*BT + b, k*KP + p]: per-(partition, batch-tile)
    loads are fully contiguous per partition.
    """
    return np.ascontiguousarray(
        xc.astype(MM_NP).reshape(NBT, BT, KCH, KP).transpose(3, 0, 2, 1)
    )


def kernel(x, conv_w, w1, b1, w2, b2):
    x = np.asarray(x, dtype=np.float32)
    w1e_r, b1_r, w2_r = _host_prep_weights(
        np.asarray(conv_w, np.float32),
        np.asarray(w1, np.float32),
        np.asarray(b1, np.float32),
        np.asarray(w2, np.float32),
    )
    b2 = np.asarray(b2, np.float32).reshape(1, O)

    if "nc" not in _cache:
        _cache["nc"] = _build_nc()
    nc = _cache["nc"]

    in_maps = []
    for c in range(N_CORES):
        xc = x[c * BS : (c + 1) * BS]  # [BS, 784]
        in_maps.append(
            {"xt": _host_prep_x(xc), "w1e": w1e_r, "b1r": b1_r, "w2r": w2_r}
        )

    res = run_bass_kernel_spmd(nc, in_maps, list(range(N_CORES)))

    y = np.empty((B, O), dtype=np.float32)
    for c in range(N_CORES):
        ytp = res.results[c]["ytp"]  # [YP_P, BS]
        if L2_COLTILE:
            yc = ytp[PB[0] : PB[0] + O]
            for j in range(1, MCH):
                yc = yc + ytp[PB[j] : PB[j] + O]
        else:
            yc = ytp[0:O]
        y[c * BS : (c + 1) * BS] = yc.T + b2
    return y


# revision 21
# speedup vs baseline: 1.0382x; 1.0382x over previous
"""Trainium2 Bass kernel for DigitConvolutionalModel.

Model: x[B,784] -> reshape 28x28 -> 3x3 valid conv -> [B,676] -> FC(676,300)
       -> ReLU -> FC(300,10).

Strategy:
  * Fold the conv into FC1 on the host: feat @ w1 == x @ W1e where
    W1e[784,300] = C @ w1 (C = sparse conv scatter). Weight-only preprocessing.
  * Pure data parallel over 8 NeuronCores: batch shard of 8192 rows per core.
  * Per-core shard is passed pre-transposed (feature-major) so the contraction
    dim (784 = 7 chunks x 112) sits on SBUF partitions; the kernel computes
    transposed activations throughout (batch on the free axis):
        a1T[300,b] = relu(W1e.T @ xT + b1);  yT[10,b] = w2.T @ a1T
  * fp16 matmul operands (1 cyc/row at every PE p-state, half the HBM
    traffic of fp32) with fp32 PSUM accumulation; biases fp32.
  * SUBT=2 batch tiles are processed per weight-stationary step: each
    LDWEIGHTS serves SUBT matmuls (measured ~25 ns/MM less PE time than
    reloading weights per matmul).
  * Layer 2 is column-tiled: the three K=100 hidden chunks run as
    concurrent matmuls in PE column groups (0,32,64); the three partial
    yT outputs land on disjoint PSUM partitions (0-9, 32-41, 64-73), are
    copied to SBUF as one [74,BT] tile, stored with one DMA, and summed
    on the host during the gather step (b2 is added there too).
  * Layer 2 for a group is emitted between L1 j-chunks of the next group
    so the PE never waits on the ReLU (no PE gaps, stays at max p-state).
  * Output: ytp[74,8192] per core; host takes rows {0-9,32-41,64-73}.
"""

import os
import sys

sys.path.insert(0, "/opt/trn_rl_repo")

import numpy as np

import concourse.tile as tile
from concourse import bacc, mybir
from concourse.bass_utils import run_bass_kernel_spmd

# ---- problem constants (hardcoded per harness contract) ----
B = 65536
D = 784  # 28*28
H = 300
O = 10
IMG = 28
KH = KW = 3
OUT_HW = IMG - KH + 1  # 26

N_CORES = 8
BS = B // N_CORES  # 8192 rows per core

KCH = 7  # contraction chunks
KP = D // KCH  # 112 partitions per chunk
BT = int(os.environ.get("BT_SIZE", "512"))  # batch tile (512 = one PSUM bank)
NBT = BS // BT
MPAD = 128  # padded partition count for hidden-chunk tensors

# matmul operand dtype: f16 (default) | bf16 | f32r | f32
_MM_CHOICE = os.environ.get("BASS_MM_DT", "f16")
MM_DT = {
    "f32": mybir.dt.float32,
    "f32r": mybir.dt.float32r,
    "bf16": mybir.dt.bfloat16,
}.get(_MM_CHOICE, mybir.dt.float16)
if _MM_CHOICE in ("f32", "f32r"):
    MM_NP = np.float32
elif _MM_CHOICE == "bf16":
    import ml_dtypes

    MM_NP = ml_dtypes.bfloat16
else:
    MM_NP = np.float16

# hidden-dim chunking (sum must be H)
M_CHUNKS = {
    "even": [100, 100, 100],
    "fwl": [128, 128, 44],
}[os.environ.get("M_CHUNKS", "even")]
M_OFFS = [sum(M_CHUNKS[:i]) for i in range(len(M_CHUNKS))]
MCH = len(M_CHUNKS)

# tunables (env-overridable for experiments)
SUBT = int(os.environ.get("SUBT", "2"))  # batch tiles per weight-stationary step
XP_BUFS = int(os.environ.get("XP_BUFS", "5"))
AP_BUFS = int(os.environ.get("AP_BUFS", "4"))
PS1_BUFS = int(os.environ.get("PS1_BUFS", "3" if SUBT == 2 else "4"))
PS2_BUFS = int(os.environ.get("PS2_BUFS", "2"))
X_DMA_SPLIT = int(os.environ.get("X_DMA_SPLIT", "1"))  # k-chunk granularity of x loads
L2_COLTILE = os.environ.get("L2_COLTILE", "1") == "1"
L2_PIPELINE = os.environ.get("L2_PIPELINE", "1") == "1"
REPS = int(os.environ.get("KERNEL_REPS", "1"))  # timing only: repeat body in-module
SKIP_L2 = os.environ.get("SKIP_L2", "0") == "1"  # ablation: L2 on first group only
SKIP_X = os.environ.get("SKIP_X", "0") == "1"  # ablation: load x once, reuse
ACT_ENG = os.environ.get("ACT_ENG", "act")  # act | dve | pool | mix (relu engine)
YT_ENG = os.environ.get("YT_ENG", "dve")  # dve | pool | act (psum->sbuf copy)
X_DMA_ENG = os.environ.get("X_DMA_ENG", "sync")  # engine issuing x loads
YT_DMA_ENG = os.environ.get("YT_DMA_ENG", "sync")  # engine issuing yt stores
ACT_TINY = os.environ.get("ACT_TINY", "0") == "1"  # ablation: 1-partition act
YT_F16 = os.environ.get("YT_F16", "1") == "1"  # store y partials as f16

# partial-output partition bases (column groups) when col-tiling
PB = [32 * j for j in range(MCH)] if L2_COLTILE else [0] * MCH
YP_P = PB[-1] + O if L2_COLTILE else O  # partitions used by ps2/yt tiles

_cache = {}


def _build_nc():
    f32 = mybir.dt.float32
    mdt = MM_DT

    nc = bacc.Bacc("TRN2", target_bir_lowering=False, debug=False, num_devices=N_CORES)
    xt_d = nc.declare_dram_parameter("xt", [KP, NBT, KCH, BT], mdt, isOutput=False)
    w1_d = nc.declare_dram_parameter("w1e", [KP, KCH * H], mdt, isOutput=False)
    b1_d = nc.declare_dram_parameter("b1r", [MPAD, MCH], f32, isOutput=False)
    w2_d = nc.declare_dram_parameter("w2r", [MPAD, MCH * O], mdt, isOutput=False)
    yt_dt = mybir.dt.float16 if YT_F16 else f32
    ytp_d = nc.declare_dram_parameter("ytp", [YP_P, BS], yt_dt, isOutput=True)

    with tile.TileContext(nc) as tc:
        with (
            tc.tile_pool(name="singles", bufs=1) as singles,
            tc.tile_pool(name="xp", bufs=XP_BUFS) as xp,
            tc.tile_pool(name="ap", bufs=AP_BUFS) as ap,
            tc.tile_pool(name="yp", bufs=3) as yp,
            tc.tile_pool(name="ps1", bufs=PS1_BUFS, space="PSUM") as ps1p,
            tc.tile_pool(name="ps2", bufs=PS2_BUFS, space="PSUM") as ps2p,
        ):
            w1sb = singles.tile([KP, KCH * H], mdt)
            nc.sync.dma_start(w1sb[:], w1_d[:])
            b1sb = singles.tile([MPAD, MCH], f32)
            nc.sync.dma_start(b1sb[:], b1_d[:])
            w2sb = singles.tile([MPAD, MCH * O], mdt)
            nc.sync.dma_start(w2sb[:], w2_d[:])

            def _eng(name):
                return {
                    "sync": nc.sync,
                    "scalar": nc.scalar,
                    "vector": nc.vector,
                    "pool": nc.gpsimd,
                }[name]

            def load_x(bt, s, split):
                xt = xp.tile([KP, KCH, BT], mdt, name=f"xt{s}")
                step = (KCH + split - 1) // split if split > 1 else KCH
                for lo in range(0, KCH, step):
                    hi = min(lo + step, KCH)
                    _eng(X_DMA_ENG).dma_start(xt[:, lo:hi, :], xt_d[:, bt, lo:hi, :])
                return xt

            def layer2_store(a1, bt):
                ps2 = ps2p.tile([YP_P, BT], f32)
                for j in range(MCH):
                    mlen = M_CHUNKS[j]
                    if L2_COLTILE:
                        nc.tensor.matmul(
                            ps2[PB[j] : PB[j] + O, :],
                            w2sb[0:mlen, j * O : (j + 1) * O],
                            a1[0:mlen, j, :],
                            start=True,
                            stop=True,
                            tile_position=(0, PB[j]),
                        )
                    else:
                        nc.tensor.matmul(
                            ps2[:],
                            w2sb[0:mlen, j * O : (j + 1) * O],
                            a1[0:mlen, j, :],
                            start=(j == 0),
                            stop=(j == MCH - 1),
                        )
                yt = yp.tile([YP_P, BT], yt_dt)
                yt_eng = {"dve": nc.vector, "pool": nc.gpsimd, "act": nc.scalar}[
                    YT_ENG
                ]
                if YT_ENG == "act":
                    yt_eng.copy(yt[:], ps2[:])
                else:
                    yt_eng.tensor_scalar_add(yt[:], ps2[:], 0.0)
                _eng(YT_DMA_ENG).dma_start(ytp_d[:, bt * BT : (bt + 1) * BT], yt[:])

            order = [i for _ in range(REPS) for i in range(NBT)]
            groups = [order[i : i + SUBT] for i in range(0, len(order), SUBT)]
            pending = []
            xts_cache = None
            for gi, bts in enumerate(groups):
                if SKIP_X and xts_cache is not None:
                    xts = xts_cache
                else:
                    xts = [
                        load_x(bt, s, split=(KCH if gi == 0 else X_DMA_SPLIT))
                        for s, bt in enumerate(bts)
                    ]
                    xts_cache = xts
                a1s = [
                    ap.tile([MPAD, MCH, BT], mdt, name=f"a1{s}")
                    for s in range(len(bts))
                ]
                for j in range(MCH):
                    mlen, moff = M_CHUNKS[j], M_OFFS[j]
                    pss = [
                        ps1p.tile([MPAD, BT], f32, name=f"ps{s}")
                        for s in range(len(bts))
                    ]
                    for k in range(KCH):
                        for s in range(len(bts)):
                            nc.tensor.matmul(
                                pss[s][0:mlen, :],
                                w1sb[:, k * H + moff : k * H + moff + mlen],
                                xts[s][:, k, :],
                                start=(k == 0),
                                stop=(k == KCH - 1),
                            )
                    for s in range(len(bts)):
                        if ACT_TINY:
                            nc.scalar.activation(
                                a1s[s][0:1, j, :],
                                pss[s][0:1, :],
                                mybir.ActivationFunctionType.Relu,
                            )
                            continue
                        eng = ACT_ENG
                        if eng == "mix":
                            eng = "act" if (j * SUBT + s) % 2 == 0 else "dve"
                        if eng == "act":
                            nc.scalar.activation(
                                a1s[s][0:mlen, j, :],
                                pss[s][0:mlen, :],
                                mybir.ActivationFunctionType.Relu,
                                bias=b1sb[0:mlen, j : j + 1],
                            )
                        else:
                            e = nc.vector if eng == "dve" else nc.gpsimd
                            e.tensor_scalar(
                                a1s[s][0:mlen, j, :],
                                pss[s][0:mlen, :],
                                b1sb[0:mlen, j : j + 1],
                                0.0,
                                mybir.AluOpType.add,
                                mybir.AluOpType.max,
                            )
                    if j == 0 and pending and L2_PIPELINE:
                        for p in pending:
                            layer2_store(*p)
                        pending = []
                if SKIP_L2 and gi > 0:
                    pass
                elif L2_PIPELINE:
                    pending = [(a1s[s], bts[s]) for s in range(len(bts))]
                else:
                    for s in range(len(bts)):
                        layer2_store(a1s[s], bts[s])
            for p in pending:
                layer2_store(*p)

    nc.compile()
    return nc


def _host_prep_weights(conv_w, w1, b1, w2):
    # Fold conv into FC1: W1e = C @ w1, computed in f64 then cast.
    w1g = w1.astype(np.float64).reshape(OUT_HW, OUT_HW, H)
    w1e = np.zeros((IMG, IMG, H), dtype=np.float64)
    cw = conv_w.astype(np.float64)
    for di in range(KH):
        for dj in range(KW):
            w1e[di : di + OUT_HW, dj : dj + OUT_HW, :] += cw[di, dj] * w1g
    w1e = w1e.reshape(D, H).astype(np.float32)

    w1e_r = np.ascontiguousarray(
        w1e.reshape(KCH, KP, H).transpose(1, 0, 2).reshape(KP, KCH * H)
    ).astype(MM_NP)
    b1f = b1.reshape(H)
    b1_r = np.zeros((MPAD, MCH), np.float32)
    w2_r = np.zeros((MPAD, MCH * O), MM_NP)
    for j in range(MCH):
        mlen, moff = M_CHUNKS[j], M_OFFS[j]
        b1_r[0:mlen, j] = b1f[moff : moff + mlen]
        w2_r[0:mlen, j * O : (j + 1) * O] = w2[moff : moff + mlen, :]
    return w1e_r, b1_r, w2_r


def _host_prep_x(xc):
    """Per-core shard [BS, 784] -> feature-major DRAM layout.

    xt[p, bt, k, b] = xc[bt*BT + b, k*KP + p]: per-(partition, batch-tile)
    loads are fully contiguous per partition.
    """
    return np.ascontiguousarray(
        xc.astype(MM_NP).reshape(NBT, BT, KCH, KP).transpose(3, 0, 2, 1)
    )


def kernel(x, conv_w, w1, b1, w2, b2):
    x = np.asarray(x, dtype=np.float32)
    w1e_r, b1_r, w2_r = _host_prep_weights(
        np.asarray(conv_w, np.float32),
        np.asarray(w1, np.float32),
        np.asarray(b1, np.float32),
        np.asarray(w2, np.float32),
    )
    b2 = np.asarray(b2, np.float32).reshape(1, O)

    if "nc" not in _cache:
        _cache["nc"] = _build_nc()
    nc = _cache["nc"]

    in_maps = []
    for c in range(N_CORES):
        xc = x[c * BS : (c + 1) * BS]  # [BS, 784]
        in_maps.append(
            {"xt": _host_prep_x(xc), "w1e": w1e_r, "b1r": b1_r, "w2r": w2_r}
        )

    res = run_bass_kernel_spmd(nc, in_maps, list(range(N_CORES)))

    y = np.empty((B, O), dtype=np.float32)
    for c in range(N_CORES):
        ytp = res.results[c]["ytp"].astype(np.float32)  # [YP_P, BS]
        if L2_COLTILE:
            yc = ytp[PB[0] : PB[0] + O]
            for j in range(1, MCH):
                yc = yc + ytp[PB[j] : PB[j] + O]
        else:
            yc = ytp[0:O]
        y[c * BS : (c + 1) * BS] = yc.T + b2
    return y
